# revision 8
# baseline (speedup 1.0000x reference)
"""Trainium2 Bass kernel for masked-row linspace replacement.

Op: for each batch b and each idx in masked_indices[b], replace
patches[b, idx, :] with linspace(patches[b, idx, 0], patches[b, idx, -1], L).

Duplicate indices produce identical replacement rows (computed from the
original patches), so the scatter is equivalent to a per-row masked blend:
    out[r, l] = mask[r] ? (p0[r] + t[l] * (pL[r] - p0[r])) : patches[r, l]

Strategy: pure data parallel over batch across 8 cores. On each core,
rows are processed in chunks of 128 (partition dim = row, free dim = l).
Per chunk: one tensor_scalar computes lin = t*(pL-p0) + p0 (2x DVE mode),
then one copy_predicated per 32-chunk group overwrites masked rows in the
loaded tile, which is stored back out.
"""

import os
import numpy as np

B, N, L = 256, 1024, 128
NCORES = 8
BPC = B // NCORES          # 32 batches per core
R = BPC * N                # 32768 rows per core
P = 128                    # rows per chunk (partition dim)
CHUNKS = R // P            # 256 chunks per core
GROUPS = 8                 # tile groups per core
CPG = CHUNKS // GROUPS     # 32 chunks per group (2 MiB tiles)

_built = None
LAST_RESULT = None


def _build_module():
    global _built
    if _built is not None:
        return _built
    import concourse.bass as bass
    import concourse.mybir as mybir
    from concourse.tile import TileContext

    f32 = mybir.dt.float32
    nc = bass.Bass()
    x = nc.declare_dram_parameter("x", [R, L], f32, isOutput=False)
    mk = nc.declare_dram_parameter("mk", [P, CHUNKS], mybir.dt.uint8, isOutput=False)
    tb = nc.declare_dram_parameter("tb", [P, L], f32, isOutput=False)
    out = nc.declare_dram_parameter("out", [R, L], f32, isOutput=True)

    xg = x.rearrange("(g c p) l -> g p c l", p=P, c=CPG)
    og = out.rearrange("(g c p) l -> g p c l", p=P, c=CPG)

    with TileContext(nc) as tc:
        with tc.tile_pool(name="constp", bufs=1) as constp, \
             tc.tile_pool(name="xp", bufs=8) as xp, \
             tc.tile_pool(name="yp", bufs=2) as yp, \
             tc.tile_pool(name="sp", bufs=2) as sp:
            mt = constp.tile([P, CHUNKS], mybir.dt.uint8, name="mt")
            nc.gpsimd.dma_start(out=mt, in_=mk[:, :])
            tt = constp.tile([P, L], f32, name="tt")
            nc.gpsimd.dma_start(out=tt, in_=tb[:, :])
            # The walrus codegen allows very few sync-wait commands per
            # DVE instruction, so the schedule is arranged so every
            # instruction needs at most ONE wait: dedicated tiny "absorber"
            # copies observe each DMA completion / same-engine RAW first.
            scrD = sp.tile([P, 1], f32, tag="scr", name="scrD", bufs=1)
            scrC = sp.tile([P, 2], f32, tag="scrC", name="scrC", bufs=1)
            scrM = sp.tile([P, 1], mybir.dt.uint8, tag="scrM", name="scrM", bufs=1)
            for g in range(GROUPS):
                X = xp.tile([P, CPG * L], f32, tag="X", name=f"X{g}")
                X3 = X.rearrange("p (c l) -> p c l", l=L)
                nc.gpsimd.dma_start(out=X3, in_=xg[g])
                Y = yp.tile([P, CPG * L], f32, tag="Y", name=f"Y{g}")
                Y3 = Y.rearrange("p (c l) -> p c l", l=L)
                D = sp.tile([P, CPG], f32, tag="D", name=f"D{g}")
                # D[:, c] = pL - p0 for each of the 32 chunks in this group
                # (first reader of X: absorbs the X load-DMA wait)
                nc.vector.tensor_sub(D, X3[:, :, L - 1], X3[:, :, 0])
                # absorb the same-engine RAW-completion wait on D
                nc.vector.tensor_copy(scrD, D[:, 0:1])
                if g == 0:
                    # absorb the tb / mk constant-load DMA waits
                    nc.vector.tensor_copy(scrC, tt[:, 0:2])
                    nc.vector.tensor_copy(scrM, mt[:, 0:1])
                for c in range(CPG):
                    # lin = t * D + p0, per-partition scalars (2x DVE mode)
                    nc.vector.tensor_scalar(
                        Y3[:, c, :],
                        tt[:, :],
                        D[:, c:c + 1],
                        X3[:, c, 0:1],
                        mybir.AluOpType.mult,
                        mybir.AluOpType.add,
                    )
                for c in range(CPG):
                    k = g * CPG + c
                    # mt holds the INVERTED mask: copy the original row from
                    # X over the lin values wherever the row is NOT masked.
                    mbc = mt[:, k:k + 1].broadcast_to((P, L))
                    nc.vector.copy_predicated(Y3[:, c, :], mbc, X3[:, c, :])
                nc.sync.dma_start(out=og[g], in_=Y3)

    # This walrus codegen allows very few sync commands per instruction.
    # Split any instruction carrying >1 wait into a chain of single-wait
    # NOPs on the same engine (the sequencer blocks on each in order).
    nopn = 0
    for fn in nc.m.functions:
        for bb in fn.blocks:
            newlist = []
            for inst in bb.instructions:
                si = getattr(inst, "sync_info", None)
                waits = list(si.on_wait) if si is not None and si.on_wait else []
                if len(waits) > 1:
                    for w in waits[:-1]:
                        nopn += 1
                        newlist.append(mybir.InstNoOp(
                            name=f"waitnop-{nopn}",
                            engine=inst.engine,
                            ins=[], outs=[],
                            sync_info=mybir.SyncInfo(on_wait=[w], on_update=[]),
                        ))
                    si.on_wait = waits[-1:]
                newlist.append(inst)
            bb.instructions[:] = newlist
    _built = nc
    return nc


def _host_inputs(patches, masked_indices):
    patches = np.ascontiguousarray(np.asarray(patches, dtype=np.float32))
    idx = np.asarray(masked_indices).astype(np.int64)
    invm = np.ones((B, N), dtype=np.uint8)
    invm[np.arange(B)[:, None], idx] = 0
    t = np.arange(L, dtype=np.float32) / np.float32(L - 1)
    tbuf = np.ascontiguousarray(np.broadcast_to(t, (P, L)))
    in_maps = []
    for i in range(NCORES):
        shard = patches[i * BPC:(i + 1) * BPC].reshape(R, L)
        m = invm[i * BPC:(i + 1) * BPC].reshape(CHUNKS, P).T
        in_maps.append({
            "x": np.ascontiguousarray(shard),
            "mk": np.ascontiguousarray(m),
            "tb": tbuf,
        })
    return in_maps


def kernel(patches, masked_indices):
    global LAST_RESULT
    from concourse.bass_utils import run_bass_kernel_spmd

    nc = _build_module()
    in_maps = _host_inputs(patches, masked_indices)
    trace = bool(os.environ.get("BASS_KERNEL_TRACE"))
    res = run_bass_kernel_spmd(nc, in_maps, list(range(NCORES)), trace=trace)
    LAST_RESULT = res
    outs = [res.results[i]["out"].reshape(BPC, N, L) for i in range(NCORES)]
    return np.concatenate(outs, axis=0)
